# revision 1
# baseline (speedup 1.0000x reference)
"""Trainium2 Bass kernel for nn_DFVAE (3-stage MoE routing with sorted ids).

Strategy (hardcoded for N=16384, LD=512, experts (8, 6, 16), 8 cores):
  - Data-parallel: core c owns rows [2048c, 2048(c+1)).
  - Activations kept feature-major ([LD, rows]) and SBUF-resident across all
    three stages; z is pre-transposed on host, output transposed back on host.
  - Sorted ids => each expert owns a contiguous row segment. Every (core,
    stage) is a list of 512-row windows, each lying inside one expert piece;
    ragged tails use overlap-shifted windows (idempotent rewrites). Short
    shard-edge pieces are emitted FIRST so later in-piece windows overwrite
    the rows they wrongly touched (ACT executes writes in program order).
  - Per window: the expert's weights+bias live as one host-packed 8208B row
    per partition, fetched with four element_offset indirect-DMA gathers
    (data-driven routing, uniform SPMD program, minimal gpsimd descriptor
    generation); the row offset is loaded into PE/ACT registers and used as
    a dynamic AP offset so matmuls read the activation tile directly.
  - Matmuls in float32r (full PE rate at N>=256, ~1.5e-4 rel err per stage);
    activations stored f32r end-to-end (one rounding per stage).
    MOE_MM_DTYPE=float32 gives exact fp32 (4x slower); MOE_DYN_RHS=0 falls
    back to a DVE staging copy for the matmul rhs.
"""
import os

import numpy as np

import concourse.bass as bass
import concourse.mybir as mybir
import concourse.tile as tile
from concourse import bacc, bass_utils
from concourse.bass import ds, ts

N = 16384
LD = 512
NCORES = 8
SH = N // NCORES  # 2048 rows per core
WIN = 512
P = 128
KO = LD // P  # 4 k/m subtiles
STAGE_E = (8, 6, 16)

LAST_RESULTS = None  # test harness reads exec_time_ns off this

_program_cache = {}


def _segments(ids):
    starts = np.flatnonzero(np.diff(ids, prepend=-1))
    ends = np.append(starts[1:], len(ids))
    return list(zip(starts.tolist(), ends.tolist(), ids[starts].tolist()))


def _windows_for_core(segs, lo, hi, win):
    """(row_start, expert) windows covering [lo, hi); short edge pieces first.

    Correctness invariant (checked by caller via _legal_win): every piece
    shorter than `win` must touch a shard edge, and its wrongly-overwritten
    neighbor rows are covered by the neighbor piece's own windows, which are
    emitted later (ACT writes execute in program order)."""
    short, norm = [], []
    for a0, b0, e in segs:
        a, b = max(a0, lo), min(b0, hi)
        if a >= b:
            continue
        length = b - a
        if length < win:
            if a == lo:
                short.append((lo, e))
            elif b == hi:
                short.append((hi - win, e))
            else:
                raise AssertionError(f"interior short piece [{a},{b})")
        else:
            for i in range(length // win):
                norm.append((a + i * win, e))
            if length % win:
                norm.append((b - win, e))
    return short + norm


def _legal_win(segs, win):
    """A window size is legal if on every shard, all interior (non-shard-edge)
    pieces are >= win (edge pieces of any size are fixed up by emit order)."""
    for c in range(NCORES):
        lo, hi = c * SH, (c + 1) * SH
        for a0, b0, _ in segs:
            a, b = max(a0, lo), min(b0, hi)
            if a >= b:
                continue
            if (b - a) < win and a != lo and b != hi:
                return False
    return True


def _build_program(C, WINS, mm_dtype_name, dyn_rhs=False, n_queues=1):
    nc = bacc.Bacc("TRN2", target_bir_lowering=False, debug=False,
                   enable_asserts=False, num_devices=NCORES,
                   num_swdge_queues=n_queues)

    def _retag_last_dma(qi):
        # move the just-issued dynamic DMA onto SWDGE queue qi; safe because
        # Tile's FIFO-based sem elision is disabled, so every dependency is an
        # explicit semaphore that travels with the instruction
        name = list(nc.inst_map)[-1]
        inst = nc.inst_map[name]
        assert type(inst).__name__ == "InstDMACopy", type(inst).__name__
        inst.queue = f"qPoolDynamic{qi or ''}"
    f32 = mybir.dt.float32
    i32 = mybir.dt.int32
    mmdt = getattr(mybir.dt, mm_dtype_name)
    ACT = mybir.EngineType.Activation
    DVE = mybir.EngineType.DVE
    PE = mybir.EngineType.PE
    Ctot = sum(C)

    # With dyn_rhs, activations live as mmdt (float32r) end-to-end: the ACT
    # writes round to f32r and matmuls read the big tile directly at a dynamic
    # offset (no DVE staging copy). Same one-rounding-per-stage numerics.
    act_dt = mmdt if dyn_rhs else f32
    WROW = KO * LD + KO  # per-partition packed row: 4x512 weights + 4 biases
    zT = nc.dram_tensor("zT_shard", [LD, SH], act_dt, kind="ExternalInput").ap()
    # host-packed per-expert rows: Wb[s][e*128+p] = [W[e][0*128+p? see host] , b]
    Wbt = [
        nc.dram_tensor(f"Wb_{s}", [STAGE_E[s] * P, WROW], f32, kind="ExternalInput").ap()
        for s in range(3)
    ]
    widx_t = nc.dram_tensor("widx", [P, Ctot], i32, kind="ExternalInput").ap()
    rowoff_t = nc.dram_tensor("rowoff", [1, 2 * Ctot], i32, kind="ExternalInput").ap()
    outT = nc.dram_tensor("outT", [LD, SH], act_dt, kind="ExternalOutput").ap()

    stage_of_slot = []
    for s in range(3):
        stage_of_slot += [s] * C[s]

    with tile.TileContext(nc) as tc:
        with (
            tc.tile_pool(name="const", bufs=1) as cpool,
            tc.tile_pool(name="w", bufs=3) as wpool,
            tc.tile_pool(name="zwin", bufs=3) as zwpool,
            tc.tile_pool(name="yt", bufs=1) as ytpool,
            tc.tile_pool(name="psum", bufs=8, space="PSUM") as ppool,
        ):
            zt_sb = cpool.tile([P, KO, SH], act_dt)
            nc.sync.dma_start(zt_sb[:], zT.rearrange("(ko p) r -> p ko r", p=P))
            desc_sb = cpool.tile([1, 2 * Ctot], i32)
            nc.sync.dma_start(desc_sb[:], rowoff_t)
            widx_sb = cpool.tile([P, Ctot], i32)
            nc.sync.dma_start(widx_sb[:], widx_t)

            # one activation buffer per stage boundary (no ping-pong reuse):
            # removes WAR edges that serialize a stage's writes against the
            # previous stage's conservatively-tracked dynamic reads
            stage_bufs = [zt_sb] + [
                ytpool.tile([P, KO, SH], act_dt, tag=f"act{i}", name=f"act{i}")
                for i in range(1, 4)
            ]
            for slot in range(Ctot):
                s = stage_of_slot[slot]
                cur, nxt = stage_bufs[s], stage_bufs[s + 1]
                # two gathers per slot over the packed 8208B rows (k01 | k23+bias):
                # halves gpsimd descriptor generation vs 4 gathers while keeping
                # matmul deps reasonably fine-grained.
                # expert weights+bias via two register-offset DMAs over the
                # packed rows: ~2x less gpsimd descriptor-generation than
                # per-row indirect gathers, while keeping matmul deps 2-way
                e_val = nc.values_load(
                    desc_sb[0:1, Ctot + slot : Ctot + slot + 1],
                    engines=[mybir.EngineType.Pool],
                    min_val=0, max_val=STAGE_E[s] - 1,
                    skip_runtime_bounds_check=True,
                )
                w_sb = wpool.tile([P, WROW], mmdt, tag="w")
                Wv = Wbt[s].rearrange("(e p) c -> e p c", p=P)
                for g in range(2):
                    lo = g * 2 * LD
                    hi = WROW if g else 2 * LD
                    nc.gpsimd.dma_start(
                        w_sb[:, lo:hi],
                        Wv[ds(e_val, 1), :, lo:hi].rearrange("e p c -> p (e c)"),
                    )
                    if n_queues > 1:
                        _retag_last_dma(g % n_queues)
                win = WINS[s]
                r_val = nc.values_load(
                    desc_sb[0:1, slot : slot + 1],
                    engines=[PE, ACT, DVE] if dyn_rhs else [DVE, ACT],
                    min_val=0,
                    max_val=SH - win,
                    skip_runtime_bounds_check=True,
                )
                if not dyn_rhs:
                    zwin = zwpool.tile([P, KO, win], mmdt, tag="zwin")
                    nc.vector.tensor_copy(out=zwin[:], in_=cur[:, :, ds(r_val, win)])
                chunks = [WIN] * (win // WIN) + ([win % WIN] if win % WIN else [])
                off = 0
                for sz in chunks:
                    for m in range(KO):
                        psum = ppool.tile([P, WIN], f32, tag="ps")
                        for k in range(KO):
                            nc.tensor.matmul(
                                psum[:, :sz],
                                lhsT=w_sb[:, k * LD + m * P : k * LD + (m + 1) * P],
                                rhs=cur[:, k, ds(r_val + off, sz)] if dyn_rhs
                                else zwin[:, k, off : off + sz],
                                start=(k == 0),
                                stop=(k == KO - 1),
                            )
                        bias_ap = w_sb[:, KO * LD + m : KO * LD + m + 1].bitcast(f32)
                        if dyn_rhs and m % 2 == 1:
                            # relu(psum + b) on DVE: (psum + b) max 0 —
                            # splits PSUM evacuation across ACT and DVE
                            nc.vector.tensor_scalar(
                                nxt[:, m, ds(r_val + off, sz)],
                                psum[:, :sz],
                                bias_ap,
                                0.0,
                                mybir.AluOpType.add,
                                mybir.AluOpType.max,
                            )
                        else:
                            nc.scalar.activation(
                                nxt[:, m, ds(r_val + off, sz)],
                                psum[:, :sz],
                                mybir.ActivationFunctionType.Relu,
                                bias=bias_ap,
                            )
                    off += sz
            nc.sync.dma_start(outT.rearrange("(ko p) r -> p ko r", p=P), stage_bufs[3][:])
    nc.compile()
    return nc


def _kernel_numpy_fallback(z, Ws, bs, ids_all):
    out = np.asarray(z, np.float32)
    for s in range(3):
        nxt = np.empty_like(out)
        ids = ids_all[s]
        for e in range(Ws[s].shape[0]):
            mask = ids == e
            if mask.any():
                nxt[mask] = np.maximum(out[mask] @ Ws[s][e] + bs[s][e], 0.0)
        out = nxt
    return out


def kernel(z, W_dataset, b_dataset, W_assay, b_assay, W_donor, b_donor,
           dataset_ids, assay_ids, donor_ids):
    global LAST_RESULTS
    mm_dtype_name = os.environ.get("MOE_MM_DTYPE", "float32r")

    ids_all = [
        np.asarray(dataset_ids, np.int32),
        np.asarray(assay_ids, np.int32),
        np.asarray(donor_ids, np.int32),
    ]
    Ws = [
        np.ascontiguousarray(np.asarray(W_dataset, np.float32)),
        np.ascontiguousarray(np.asarray(W_assay, np.float32)),
        np.ascontiguousarray(np.asarray(W_donor, np.float32)),
    ]
    bs = [
        np.asarray(b_dataset, np.float32),
        np.asarray(b_assay, np.float32),
        np.asarray(b_donor, np.float32),
    ]
    zT = np.ascontiguousarray(np.asarray(z, np.float32).T)  # [LD, N]

    if any(np.any(np.diff(ids) < 0) for ids in ids_all):
        return _kernel_numpy_fallback(z, Ws, bs, ids_all)
    try:
        segs_all = [_segments(ids_all[s]) for s in range(3)]

        def _pick_win(segs):
            # choose the legal window minimizing padded compute (total 512-row
            # chunk count across the padded slot grid), tie-break fewer slots
            best = None
            for w in (512, 768, 1024):
                if not _legal_win(segs, w):
                    continue
                cmax = max(
                    len(_windows_for_core(segs, c * SH, (c + 1) * SH, w))
                    for c in range(NCORES)
                )
                chunks = cmax * ((w + WIN - 1) // WIN)
                key = (chunks, cmax)
                if best is None or key < best[0]:
                    best = (key, w)
            return best[1] if best else WIN

        WINS = tuple(_pick_win(segs_all[s]) for s in range(3))
        wins = [
            [_windows_for_core(segs_all[s], c * SH, (c + 1) * SH, WINS[s])
             for c in range(NCORES)]
            for s in range(3)
        ]
    except AssertionError:
        # ids not sorted / pathological segment layout: correctness fallback
        return _kernel_numpy_fallback(z, Ws, bs, ids_all)
    C = tuple(max(len(wins[s][c]) for c in range(NCORES)) for s in range(3))
    for s in range(3):
        for c in range(NCORES):
            w = wins[s][c]
            while len(w) < C[s]:
                w.append(w[-1])
    Ctot = sum(C)

    arange = np.arange(P, dtype=np.int32)
    rowoff = np.zeros((NCORES, 1, 2 * Ctot), np.int32)
    widx = np.zeros((NCORES, P, Ctot), np.int32)
    for c in range(NCORES):
        off = 0
        for s in range(3):
            for j, (r, e) in enumerate(wins[s][c]):
                slot = off + j
                rowoff[c, 0, slot] = r - c * SH
                rowoff[c, 0, Ctot + slot] = e
                widx[c, :, slot] = e * P + arange
            off += C[s]

    # packed per-(expert, partition) rows: 4x512 weight cols then 4 biases
    Wb = []
    for s in range(3):
        E = STAGE_E[s]
        w_pack = Ws[s].reshape(E, KO, P, LD).transpose(0, 2, 1, 3).reshape(E, P, KO * LD)
        b_pack = bs[s].reshape(E, KO, P).transpose(0, 2, 1)  # [E, P, KO]
        Wb.append(
            np.ascontiguousarray(
                np.concatenate([w_pack, b_pack], axis=2).reshape(E * P, KO * LD + KO)
            )
        )

    dyn_rhs = os.environ.get("MOE_DYN_RHS", "1") == "1"
    n_queues = int(os.environ.get("MOE_DMA_QUEUES", "4"))
    key = (C, WINS, mm_dtype_name, dyn_rhs, n_queues)
    if key not in _program_cache:
        _program_cache[key] = _build_program(C, WINS, mm_dtype_name, dyn_rhs, n_queues)
    nc = _program_cache[key]

    in_maps = []
    for c in range(NCORES):
        m = {
            "zT_shard": np.ascontiguousarray(zT[:, c * SH : (c + 1) * SH]),
            "rowoff": rowoff[c],
            "widx": widx[c],
        }
        for s in range(3):
            m[f"Wb_{s}"] = Wb[s]
        in_maps.append(m)

    res = bass_utils.run_bass_kernel_spmd(nc, in_maps, core_ids=list(range(NCORES)))
    LAST_RESULTS = res

    out = np.empty((N, LD), np.float32)
    for c in range(NCORES):
        out[c * SH : (c + 1) * SH] = res.results[c]["outT"].T
    return out



# revision 23
# speedup vs baseline: 1.4882x; 1.4882x over previous
"""Trainium2 Bass kernel for nn_DFVAE (3-stage MoE routing with sorted ids).

Strategy (hardcoded for N=16384, LD=512, experts (8, 6, 16), 8 cores):
  - Data-parallel: core c owns rows [2048c, 2048(c+1)).
  - bf16 activations + weights (f32 PSUM accumulate, f32 bias): ~4e-3 rel
    err end-to-end, well under the 2e-2 gate; halves all SBUF/HBM traffic
    and keeps full PE rate at any moving-dim size.
  - Sorted ids => contiguous expert pieces. Each (core, stage) is covered by
    a mixed grid of 256- and 512-row windows, each inside one piece:
    per piece floor(L/512) 512-windows + one overlap-shifted tail (256 if the
    remainder fits, else 512). Short shard-edge pieces get an edge-anchored
    spill window (min size covering the piece); every wrongly-spilled row is
    rewritten by the neighbor piece's first 512 head window because classes
    are emitted ascending (256 then 512) and shorts precede norms in-class.
    Cores pad to the per-stage class maxima with idempotent sub-window
    repeats, keeping the program uniform (SPMD).
  - Weights+biases are HOST-packed per core and DMA'd as plain static copies
    (no gpsimd gathers, no descriptor generation). Stage 0 is deduped (only
    the <=NW0 distinct experts ship early; slots pick theirs via a dynamic
    lhsT column offset) so the z stream owns the head bandwidth; stages 1/2
    are packed per slot (static offsets), streaming in under stage-0 compute.
  - Activations live as four per-k [P, SH] tiles per stage boundary; z
    streams as 4 k-chunk DMAs and stage 0 runs slot-PAIRS k-outer (8 PSUM
    banks = 2 slots) so PE consumption tracks z chunk arrival.
  - Stage-2 outputs land in small per-slot tiles written at static offsets
    and stream to per-slot OUTPUT DRAM TENSORS via independent static DMAs
    (no dynamic-dst write-write serialization); the host assembles the final
    activation in slot order, mirroring device write order.
  - Per-slot dynamic row offsets come from batched TensorLoads per engine
    (PE/ACT/DVE); m-parity fixes the eviction engine so same-row rewrites
    stay ordered (same engine => program order).
  - A tiny warm-up matmul right after the descriptor DMA starts the PE
    p-state ramp clock early, so real matmuls run at the full 2.4 GHz.
"""
import os

import numpy as np

import concourse.bass as bass
import concourse.mybir as mybir
import concourse.tile as tile
from concourse import bacc, bass_utils
from concourse.bass import ds

N = 16384
LD = 512
NCORES = 8
SH = N // NCORES  # 2048 rows per core
P = 128
KO = LD // P  # 4 k/m subtiles
STAGE_E = (8, 6, 16)
SIZES = (256, 512)
WSLOT = KO * LD  # 2048 bf16 weight cols per pack entry per partition

LAST_RESULTS = None  # test harness reads exec_time_ns off this

_program_cache = {}


def _segments(ids):
    starts = np.flatnonzero(np.diff(ids, prepend=-1))
    ends = np.append(starts[1:], len(ids))
    return list(zip(starts.tolist(), ends.tolist(), ids[starts].tolist()))


def _core_windows(segs, lo, hi):
    """Per-class windows for one shard: {sz: (spills, norms)} lists of
    (start, expert). Raises on layouts this scheme can't cover."""
    big = max(SIZES)
    out = {sz: ([], []) for sz in SIZES}
    for a0, b0, e in segs:
        a, b = max(a0, lo), min(b0, hi)
        if a >= b:
            continue
        L = b - a
        edge = a == lo or b == hi
        if L < big:
            if not edge:
                raise AssertionError(f"interior short piece [{a},{b})")
            w = min(sz for sz in SIZES if sz >= L)
            out[w][0].append((lo if a == lo else hi - w, e))
            continue
        n, rem = L // big, L % big
        for i in range(n):
            out[big][1].append((a + i * big, e))
        if rem:
            t = min(sz for sz in SIZES if sz >= rem)  # t <= big <= L
            out[t][1].append((b - t, e))
    if not out[big][1]:
        raise AssertionError("no full-size in-piece window on shard")
    return out


def make_plan(segs_all):
    """Window layout for all (stage, core): returns (slot_plan, slots) where
    slots[s][c] = [(start, expert, size), ...] in program order.

    Per stage the program slot sequence is
        [512]*(C512-H) ++ [256]*C256 ++ [512]*H
    where H bounds the per-core count of spill-CONFLICTING 512 head windows
    (512 norms overlapping a spill window's range). Early 512 slots keep the
    PE fed at full rate while z streams in; spills sit in the 256 block and
    their conflicting heads come after, preserving overwrite correctness.
    All pads are idempotent repeats (non-conflicting writes of real windows)."""
    big = max(SIZES)
    wins = [
        [_core_windows(segs_all[s], c * SH, (c + 1) * SH) for c in range(NCORES)]
        for s in range(3)
    ]
    slot_plan = []
    slots = []
    for s in range(3):
        C = {sz: max(len(wins[s][c][sz][0]) + len(wins[s][c][sz][1])
                     for c in range(NCORES)) for sz in SIZES}
        percore_cls = []
        for c in range(NCORES):
            w = wins[s][c]
            spills = [(r, e, sz) for sz in SIZES for r, e in w[sz][0]]
            big_norms = w[big][1]
            late = [
                (r, e) for r, e in big_norms
                if any(r < sr + ssz and sr < r + big for sr, _, ssz in spills)
            ]
            early = [(r, e) for r, e in big_norms if (r, e) not in late]
            small = ([(r, e) for r, e in w[256][0]] +
                     [(r, e) for r, e in w[256][1]]) if 256 in SIZES else []
            percore_cls.append((early, small, late, big_norms[-1]))
        H = max(len(late) for _, _, late, _ in percore_cls)
        n_early = C[big] - H
        plan = (big,) * n_early + (256,) * C[256] + (big,) * H
        slot_plan.append(plan)
        percore = []
        for c in range(NCORES):
            early, small, late, pad = percore_cls[c]
            # move overflow norms into the late block (allowed: only heads
            # must come after spills; norms can go anywhere)
            while len(early) > n_early:
                late.append(early.pop())
            while len(early) < n_early:
                early.append(pad)
            while len(late) < H:
                late.append(pad)
            while len(small) < C[256]:
                small.append((pad[0], pad[1]))  # 256 sub-window of pad: idem
            assert len(late) == H and len(small) == C[256]
            flat = ([(r, e, big) for r, e in early] +
                    [(r, e, 256) for r, e in small] +
                    [(r, e, big) for r, e in late])
            percore.append(flat)
        slots.append(percore)
    return tuple(slot_plan), slots


def _build_program(slot_plan, mm_dtype_name):
    nc = bacc.Bacc("TRN2", target_bir_lowering=False, debug=False,
                   enable_asserts=False, num_devices=NCORES)
    f32 = mybir.dt.float32
    i32 = mybir.dt.int32
    mmdt = getattr(mybir.dt, mm_dtype_name)
    ACT = mybir.EngineType.Activation
    DVE = mybir.EngineType.DVE
    PE = mybir.EngineType.PE
    Ctot = sum(len(p) for p in slot_plan)

    zT = nc.dram_tensor("zT_shard", [LD, SH], mmdt, kind="ExternalInput").ap()
    Wp = nc.dram_tensor("Wpack", [P, Ctot * WSLOT], mmdt, kind="ExternalInput").ap()
    Bp = nc.dram_tensor("Bpack", [P, Ctot * KO], f32, kind="ExternalInput").ap()
    desc_t = nc.dram_tensor("descs", [1, Ctot], i32, kind="ExternalInput").ap()
    outs = [
        nc.dram_tensor(f"outT_{j}", [LD, sz], mmdt, kind="ExternalOutput").ap()
        for j, sz in enumerate(slot_plan[2])
    ]

    with tile.TileContext(nc) as tc:
        with (
            tc.tile_pool(name="const", bufs=1) as cpool,
            tc.tile_pool(name="acts", bufs=1) as apool,
            tc.tile_pool(name="psum", bufs=8, space="PSUM") as ppool,
        ):
            desc_sb = cpool.tile([1, Ctot], i32)
            nc.sync.dma_start(desc_sb[:], desc_t)
            bias_sb = cpool.tile([P, Ctot * KO], f32)
            w_sb = cpool.tile([P, Ctot * WSLOT], mmdt)
            bufs = [
                [apool.tile([P, SH], mmdt, tag=f"act{s}_{k}", name=f"act{s}_{k}")
                 for k in range(KO)]
                for s in range(3)
            ]
            out_tiles = [
                apool.tile([P, KO, sz], mmdt, tag=f"out{j}", name=f"out{j}")
                for j, sz in enumerate(slot_plan[2])
            ]

            # DMA issue order = transfer order on the contended DMA engines:
            # first slot's weights, bias, the four z k-chunks (stage-0 512
            # slots consume them at matching rate), then remaining slot packs
            def _w_load(entry, n=1):
                nc.sync.dma_start(
                    w_sb[:, entry * WSLOT:(entry + n) * WSLOT],
                    Wp[:, entry * WSLOT:(entry + n) * WSLOT],
                )
            _w_load(0)
            nc.sync.dma_start(bias_sb[:], Bp)
            for k in range(KO):
                nc.sync.dma_start(bufs[0][k][:], zT[k * P:(k + 1) * P, :])
            for j in range(1, Ctot):
                _w_load(j)

            # warm-up matmul: starts the PE p-state ramp clock early so real
            # matmuls hit full clock sooner (result never read)
            warm = ppool.tile([2, 16], f32, tag="ps", name="warm")
            nc.tensor.matmul(warm[:], lhsT=desc_sb.bitcast(mmdt)[0:1, 0:2],
                             rhs=desc_sb.bitcast(mmdt)[0:1, 0:16],
                             start=True, stop=True)

            rvals = nc.values_load_multi_w_load_instructions(
                desc_sb[0:1, 0:Ctot],
                engines=[PE, ACT, DVE],
                min_val=0,
                max_val=SH - min(SIZES),
                skip_runtime_bounds_check=True,
            )[1]

            def emit_slot_matmuls(slot, sz, r, cur, k):
                """One k-step (KO matmuls) of a slot into its 4 psum banks."""
                w_col = slot * WSLOT
                for m in range(KO):
                    nc.tensor.matmul(
                        psums[slot][m][:, :sz],
                        lhsT=w_sb[:, w_col + k * LD + m * P:
                                  w_col + k * LD + (m + 1) * P],
                        rhs=cur[k][:, ds(r, sz)],
                        start=(k == 0),
                        stop=(k == KO - 1),
                    )

            def emit_evictions(slot, s, j2, sz, r, nxt):
                for m in range(KO):
                    psum = psums[slot][m]
                    bias_ap = bias_sb[:, slot * KO + m:slot * KO + m + 1]
                    dst = (nxt[m][:, ds(r, sz)] if nxt is not None
                           else out_tiles[j2][:, m, :sz])
                    if m % 2 == 1:
                        # relu(psum + b) on DVE: (psum + b) max 0; m-parity
                        # keeps each row's rewrites on one engine (ordered)
                        nc.vector.tensor_scalar(
                            dst, psum[:, :sz], bias_ap, 0.0,
                            mybir.AluOpType.add, mybir.AluOpType.max,
                        )
                    else:
                        nc.scalar.activation(
                            dst, psum[:, :sz],
                            mybir.ActivationFunctionType.Relu, bias=bias_ap,
                        )

            psums = {}
            slot = 0
            for s in range(3):
                cur = bufs[s]
                nxt = bufs[s + 1] if s < 2 else None
                plan = slot_plan[s]
                for j2, sz in enumerate(plan):
                    sl = slot + j2
                    r = nc.s_assert_within(
                        rvals[sl], min_val=0, max_val=SH - sz,
                        skip_runtime_assert=True,
                    )
                    psums[sl] = [
                        ppool.tile([P, max(SIZES)], f32, tag="ps",
                                   name=f"ps{sl}_{m}")
                        for m in range(KO)
                    ]
                    # k-outer: a slot's first KO matmuls only need the k=0
                    # activation tile (head pipelining with the z stream)
                    for k in range(KO):
                        emit_slot_matmuls(sl, sz, r, cur, k)
                    emit_evictions(sl, s, j2, sz, r, nxt)
                    if nxt is None:
                        nc.sync.dma_start(
                            outs[j2].rearrange("(ko p) c -> p ko c", p=P),
                            out_tiles[j2][:, :, :sz],
                        )
                    del psums[sl]
                slot += len(plan)
    nc.compile()
    return nc


def _kernel_numpy_fallback(z, Ws, bs, ids_all):
    out = np.asarray(z, np.float32)
    for s in range(3):
        nxt = np.empty_like(out)
        ids = ids_all[s]
        for e in range(Ws[s].shape[0]):
            mask = ids == e
            if mask.any():
                nxt[mask] = np.maximum(out[mask] @ Ws[s][e] + bs[s][e], 0.0)
        out = nxt
    return out


def kernel(z, W_dataset, b_dataset, W_assay, b_assay, W_donor, b_donor,
           dataset_ids, assay_ids, donor_ids):
    global LAST_RESULTS
    import ml_dtypes

    mm_dtype_name = os.environ.get("MOE_MM_DTYPE", "bfloat16")
    np_mmdt = dict(bfloat16=ml_dtypes.bfloat16, float32r=np.float32,
                   float32=np.float32)[mm_dtype_name]

    ids_all = [
        np.asarray(dataset_ids, np.int32),
        np.asarray(assay_ids, np.int32),
        np.asarray(donor_ids, np.int32),
    ]
    Ws = [
        np.ascontiguousarray(np.asarray(W_dataset, np.float32)),
        np.ascontiguousarray(np.asarray(W_assay, np.float32)),
        np.ascontiguousarray(np.asarray(W_donor, np.float32)),
    ]
    bs = [
        np.asarray(b_dataset, np.float32),
        np.asarray(b_assay, np.float32),
        np.asarray(b_donor, np.float32),
    ]

    if any(np.any(np.diff(ids) < 0) for ids in ids_all):
        return _kernel_numpy_fallback(z, Ws, bs, ids_all)
    try:
        segs_all = [_segments(ids_all[s]) for s in range(3)]
        slot_plan, slots = make_plan(segs_all)
    except (AssertionError, IndexError):
        return _kernel_numpy_fallback(z, Ws, bs, ids_all)

    Ctot = sum(len(p) for p in slot_plan)

    # host-packed per-core weights/biases; lhsT layout per pack entry:
    # Wl[e][p, k*LD + m*P + j] = W[e][k*P + p, m*P + j]
    Wl = [
        Ws[s].reshape(STAGE_E[s], KO, P, KO, P)
        .transpose(0, 2, 1, 3, 4).reshape(STAGE_E[s], P, WSLOT).astype(np_mmdt)
        for s in range(3)
    ]
    Bl = [np.ascontiguousarray(bs[s].reshape(STAGE_E[s], KO, P).transpose(0, 2, 1))
          for s in range(3)]  # [E, P, KO]

    descs = np.zeros((NCORES, 1, Ctot), np.int32)
    Wpack = np.empty((NCORES, P, Ctot * WSLOT), np_mmdt)
    Bpack = np.empty((NCORES, P, Ctot * KO), np.float32)
    for c in range(NCORES):
        j = 0
        for s in range(3):
            for r, e, sz in slots[s][c]:
                descs[c, 0, j] = r - c * SH
                Wpack[c, :, j * WSLOT:(j + 1) * WSLOT] = Wl[s][e]
                Bpack[c, :, j * KO:(j + 1) * KO] = Bl[s][e]
                j += 1
        assert j == Ctot

    key = (slot_plan, mm_dtype_name)
    if key not in _program_cache:
        _program_cache[key] = _build_program(slot_plan, mm_dtype_name)
    nc = _program_cache[key]

    zT = np.asarray(z, np.float32).T.astype(np_mmdt)  # [LD, N]
    in_maps = [
        {
            "zT_shard": np.ascontiguousarray(zT[:, c * SH:(c + 1) * SH]),
            "Wpack": Wpack[c],
            "Bpack": Bpack[c],
            "descs": descs[c],
        }
        for c in range(NCORES)
    ]

    res = bass_utils.run_bass_kernel_spmd(nc, in_maps, core_ids=list(range(NCORES)))
    LAST_RESULTS = res

    # assemble stage-2 windows in slot order, mirroring device write order
    out = np.empty((N, LD), np.float32)
    for c in range(NCORES):
        shard = out[c * SH:(c + 1) * SH]
        for j, (r, e, sz) in enumerate(slots[2][c]):
            shard[r - c * SH:r - c * SH + sz] = (
                res.results[c][f"outT_{j}"].astype(np.float32).T
            )
    return out


# revision 53
# speedup vs baseline: 1.5972x; 1.0732x over previous
"""Trainium2 Bass kernel for nn_DFVAE (3-stage MoE routing with sorted ids).

Strategy (hardcoded for N=16384, LD=512, experts (8, 6, 16), 8 cores):
  - Data-parallel: core c owns rows [2048c, 2048(c+1)).
  - bf16 activations + weights (f32 PSUM accumulate, f32 bias): ~4e-3 rel
    err end-to-end, well under the 2e-2 gate; halves all SBUF/HBM traffic
    and keeps full PE rate at any moving-dim size.
  - Sorted ids => contiguous expert pieces. Each (core, stage) is covered by
    a mixed grid of 256- and 512-row windows, each inside one piece:
    per piece floor(L/512) 512-windows + one overlap-shifted tail (256 if the
    remainder fits, else 512). Short shard-edge pieces get an edge-anchored
    spill window (min size covering the piece); every wrongly-spilled row is
    rewritten by the neighbor piece's first 512 head window because classes
    are emitted ascending (256 then 512) and shorts precede norms in-class.
    Cores pad to the per-stage class maxima with idempotent sub-window
    repeats, keeping the program uniform (SPMD).
  - Weights+biases are HOST-packed per core and DMA'd as plain static copies
    (no gpsimd gathers, no descriptor generation). Stage 0 is deduped (only
    the <=NW0 distinct experts ship early; slots pick theirs via a dynamic
    lhsT column offset) so the z stream owns the head bandwidth; stages 1/2
    are packed per slot (static offsets), streaming in under stage-0 compute.
  - Activations live as four per-k [P, SH] tiles per stage boundary; z
    streams as 4 k-chunk DMAs and stage 0 runs slot-PAIRS k-outer (8 PSUM
    banks = 2 slots) so PE consumption tracks z chunk arrival.
  - Stage-2 outputs land in small per-slot tiles written at static offsets
    and stream to per-slot OUTPUT DRAM TENSORS via independent static DMAs
    (no dynamic-dst write-write serialization); the host assembles the final
    activation in slot order, mirroring device write order.
  - Per-slot dynamic row offsets come from batched TensorLoads per engine
    (PE/ACT/DVE); m-parity fixes the eviction engine so same-row rewrites
    stay ordered (same engine => program order).
  - A tiny warm-up matmul right after the descriptor DMA starts the PE
    p-state ramp clock early, so real matmuls run at the full 2.4 GHz.
"""
import os

import numpy as np

import concourse.bass as bass
import concourse.mybir as mybir
import concourse.tile as tile
from concourse import bacc, bass_utils
from concourse.bass import ds

N = 16384
LD = 512
NCORES = 8
SH = N // NCORES  # 2048 rows per core
P = 128
KO = LD // P  # 4 k/m subtiles
STAGE_E = (8, 6, 16)
SIZES = (256, 512)
WSLOT = KO * LD  # 2048 bf16 weight cols per pack entry per partition

LAST_RESULTS = None  # test harness reads exec_time_ns off this

_program_cache = {}


def _segments(ids):
    starts = np.flatnonzero(np.diff(ids, prepend=-1))
    ends = np.append(starts[1:], len(ids))
    return list(zip(starts.tolist(), ends.tolist(), ids[starts].tolist()))


def _core_windows(segs, lo, hi):
    """Per-class windows for one shard: {sz: (spills, norms)} lists of
    (start, expert). Raises on layouts this scheme can't cover."""
    big = max(SIZES)
    out = {sz: ([], []) for sz in SIZES}
    for a0, b0, e in segs:
        a, b = max(a0, lo), min(b0, hi)
        if a >= b:
            continue
        L = b - a
        edge = a == lo or b == hi
        if L < big:
            if not edge:
                raise AssertionError(f"interior short piece [{a},{b})")
            w = min(sz for sz in SIZES if sz >= L)
            out[w][0].append((lo if a == lo else hi - w, e))
            continue
        n, rem = L // big, L % big
        for i in range(n):
            out[big][1].append((a + i * big, e))
        if rem:
            t = min(sz for sz in SIZES if sz >= rem)  # t <= big <= L
            out[t][1].append((b - t, e))
    if not out[big][1]:
        raise AssertionError("no full-size in-piece window on shard")
    return out


def make_plan(segs_all):
    """Window layout for all (stage, core): returns (slot_plan, slots) where
    slots[s][c] = [(start, expert, size), ...] in program order.

    Per stage the program slot sequence is
        [512]*(C512-H) ++ [256]*C256 ++ [512]*H
    where H bounds the per-core count of spill-CONFLICTING 512 head windows
    (512 norms overlapping a spill window's range). Early 512 slots keep the
    PE fed at full rate while z streams in; spills sit in the 256 block and
    their conflicting heads come after, preserving overwrite correctness.
    All pads are idempotent repeats (non-conflicting writes of real windows)."""
    big = max(SIZES)
    allow_share = os.environ.get("MOE_SHARE01", "1") == "1"
    allow_split = os.environ.get("MOE_SPLIT_TAIL", "1") == "1"
    wins = [
        [_core_windows(segs_all[s], c * SH, (c + 1) * SH) for c in range(NCORES)]
        for s in range(3)
    ]
    slot_plan = []
    slots = []
    for s in range(3):
        C = {sz: max(len(wins[s][c][sz][0]) + len(wins[s][c][sz][1])
                     for c in range(NCORES)) for sz in SIZES}
        percore_cls = []
        for c in range(NCORES):
            w = wins[s][c]
            spills = [(r, e, sz) for sz in SIZES for r, e in w[sz][0]]
            big_norms = w[big][1]
            late = [
                (r, e) for r, e in big_norms
                if any(r < sr + ssz and sr < r + big for sr, _, ssz in spills)
            ]
            early = [(r, e) for r, e in big_norms if (r, e) not in late]
            small = ([(r, e) for r, e in w[256][0]] +
                     [(r, e) for r, e in w[256][1]]) if 256 in SIZES else []
            percore_cls.append((early, small, late, big_norms[-1]))
        H = max(len(late) for _, _, late, _ in percore_cls)
        # stage 2 ends the kernel: split its final 512 window into two 256
        # halves so the tail eviction+store chain is half-sized (same rows)
        split_tail = s == 2 and H > 0 and allow_split
        plan = ((big,) * (C[big] - H) + (256,) * C[256]
                + (big,) * (H - 1 if split_tail else H)
                + ((256, 256) if split_tail else ()))
        slot_plan.append(plan)
        percore = []
        if s == 0:
            share01 = allow_share and C[big] - H >= 2
        for c in range(NCORES):
            early, small, late, pad = percore_cls[c]
            # move overflow norms into the late block (allowed: only heads
            # must come after spills; norms can go anywhere)
            while len(early) > C[big] - H:
                late.append(early.pop())
            while len(early) < C[big] - H:
                early.append(pad)
            while len(late) < H:
                late.append(pad)
            while len(small) < C[256]:
                small.append((pad[0], pad[1]))  # 256 sub-window of pad: idem
            assert len(late) == H and len(small) == C[256]
            if s == 0 and share01:
                # put two same-expert windows first: slots 0 and 1 then share
                # weight-pack entry 0, so the opening slot-PAIR runs k-outer
                # without waiting for a second weight transfer
                pair = next(
                    ((i, jj) for i in range(len(early))
                     for jj in range(i + 1, len(early))
                     if early[i][1] == early[jj][1]),
                    None,
                )
                if pair is None:
                    share01 = False
                else:
                    i, jj = pair
                    early = ([early[i], early[jj]] +
                             [w for x, w in enumerate(early) if x not in (i, jj)])
            flat = ([(r, e, big) for r, e in early] +
                    [(r, e, 256) for r, e in small] +
                    [(r, e, big) for r, e in late])
            if split_tail:
                r, e, _ = flat.pop()
                flat += [(r, e, 256), (r + 256, e, 256)]
            percore.append(flat)
        slots.append(percore)
    return tuple(slot_plan), slots, share01


def _build_program(slot_plan, share01, mm_dtype_name):
    nc = bacc.Bacc("TRN2", target_bir_lowering=False, debug=False,
                   enable_asserts=False, num_devices=NCORES)
    f32 = mybir.dt.float32
    i32 = mybir.dt.int32
    mmdt = getattr(mybir.dt, mm_dtype_name)
    ACT = mybir.EngineType.Activation
    DVE = mybir.EngineType.DVE
    PE = mybir.EngineType.PE
    Ctot = sum(len(p) for p in slot_plan)

    zT = nc.dram_tensor("zT_shard", [LD, SH], mmdt, kind="ExternalInput").ap()
    Wp = nc.dram_tensor("Wpack", [P, Ctot * WSLOT], mmdt, kind="ExternalInput").ap()
    Bp = nc.dram_tensor("Bpack", [P, Ctot * KO], f32, kind="ExternalInput").ap()
    desc_t = nc.dram_tensor("descs", [1, Ctot], i32, kind="ExternalInput").ap()
    outs = [
        nc.dram_tensor(f"outT_{j}", [LD, sz], mmdt, kind="ExternalOutput").ap()
        for j, sz in enumerate(slot_plan[2])
    ]

    with tile.TileContext(nc) as tc:
        with (
            tc.tile_pool(name="const", bufs=1) as cpool,
            tc.tile_pool(name="acts", bufs=1) as apool,
            tc.tile_pool(name="psum", bufs=8, space="PSUM") as ppool,
        ):
            desc_sb = cpool.tile([1, Ctot], i32)
            # gpsimd path: descgen on Pool, so the HWDGE descgen pipeline
            # starts on W0 immediately and the first transfers begin sooner
            if os.environ.get("MOE_DESC_GPSIMD", "1") == "1":
                nc.gpsimd.dma_start(desc_sb[:], desc_t)
            else:
                nc.sync.dma_start(desc_sb[:], desc_t)
            bias_sb = cpool.tile([P, Ctot * KO], f32)
            w_sb = cpool.tile([P, Ctot * WSLOT], mmdt)
            bufs = [
                [apool.tile([P, SH], mmdt, tag=f"act{s}_{k}", name=f"act{s}_{k}")
                 for k in range(KO)]
                for s in range(3)
            ]
            out_tiles = [
                apool.tile([P, KO, sz], mmdt, tag=f"out{j}", name=f"out{j}")
                for j, sz in enumerate(slot_plan[2])
            ]

            # DMA issue order = transfer order on the contended DMA engines:
            # the first slot-pair's weights, bias, the four z k-chunks
            # (the stage-0 pair consumes each chunk in ~1.7us >= its 1.5us
            # arrival), then the remaining slot packs stream under compute
            def _w_load(entry, n=1):
                nc.sync.dma_start(
                    w_sb[:, entry * WSLOT:(entry + n) * WSLOT],
                    Wp[:, entry * WSLOT:(entry + n) * WSLOT],
                )
            # interleave W0's k-blocks with the z k-chunks so the opening
            # pair's k-step matmuls unblock as each (weights, data) pair lands
            nc.sync.dma_start(w_sb[:, 0:LD], Wp[:, 0:LD])
            nc.sync.dma_start(bufs[0][0][:], zT[0:P, :])
            nc.sync.dma_start(w_sb[:, LD:2 * LD], Wp[:, LD:2 * LD])
            nc.sync.dma_start(bufs[0][1][:], zT[P:2 * P, :])
            nc.sync.dma_start(w_sb[:, 2 * LD:WSLOT], Wp[:, 2 * LD:WSLOT])
            for k in range(2, KO):
                nc.sync.dma_start(bufs[0][k][:], zT[k * P:(k + 1) * P, :])
            nc.sync.dma_start(bias_sb[:], Bp)
            for j in range(1, Ctot):
                if j == 1 and share01:
                    continue  # slot 1 reads entry 0; entry 1 never read
                _w_load(j)

            # warm-up matmul: starts the PE p-state ramp clock early so real
            # matmuls hit full clock sooner (result never read)
            warm = ppool.tile([2, 16], f32, tag="ps", name="warm")
            nc.tensor.matmul(warm[:], lhsT=desc_sb.bitcast(mmdt)[0:1, 0:2],
                             rhs=desc_sb.bitcast(mmdt)[0:1, 0:16],
                             start=True, stop=True)

            rvals = nc.values_load_multi_w_load_instructions(
                desc_sb[0:1, 0:Ctot],
                engines=[PE, ACT, DVE],
                min_val=0,
                max_val=SH - min(SIZES),
                skip_runtime_bounds_check=True,
            )[1]

            def emit_slot_matmuls(slot, sz, r, cur, k):
                """One k-step (KO matmuls) of a slot into its 4 psum banks."""
                w_col = 0 if (slot == 1 and share01) else slot * WSLOT
                for m in range(KO):
                    nc.tensor.matmul(
                        psums[slot][m][:, :sz],
                        lhsT=w_sb[:, w_col + k * LD + m * P:
                                  w_col + k * LD + (m + 1) * P],
                        rhs=cur[k][:, ds(r, sz)],
                        start=(k == 0),
                        stop=(k == KO - 1),
                    )

            def emit_evictions(slot, s, j2, sz, r, nxt, act_parity=0):
                for m in range(KO):
                    psum = psums[slot][m]
                    bias_ap = bias_sb[:, slot * KO + m:slot * KO + m + 1]
                    dst = (nxt[m][:, ds(r, sz)] if nxt is not None
                           else out_tiles[j2][:, m, :sz])
                    if m % 2 != act_parity:
                        # relu(psum + b) on DVE: (psum + b) max 0; m-parity
                        # keeps each row's rewrites on one engine (ordered)
                        nc.vector.tensor_scalar(
                            dst, psum[:, :sz], bias_ap, 0.0,
                            mybir.AluOpType.add, mybir.AluOpType.max,
                        )
                    else:
                        nc.scalar.activation(
                            dst, psum[:, :sz],
                            mybir.ActivationFunctionType.Relu, bias=bias_ap,
                        )

            psums = {}
            slot = 0
            for s in range(3):
                cur = bufs[s]
                nxt = bufs[s + 1] if s < 2 else None
                plan = slot_plan[s]
                # when slots 0,1 share entry 0, run them as a k-outer PAIR
                # (8 psum banks): 1.7us of PE work per z k-chunk covers the
                # ~1.5us chunk arrival, so the z stream causes no stalls
                if s == 0 and share01:
                    groups = [[0, 1]] + [[j] for j in range(2, len(plan))]
                else:
                    groups = [[j] for j in range(len(plan))]
                for group in groups:
                    rs = {}
                    for j2 in group:
                        sl = slot + j2
                        sz = plan[j2]
                        rs[j2] = nc.s_assert_within(
                            rvals[sl], min_val=0, max_val=SH - sz,
                            skip_runtime_assert=True,
                        )
                        psums[sl] = [
                            ppool.tile([P, max(SIZES)], f32, tag="ps",
                                       name=f"ps{sl}_{m}")
                            for m in range(KO)
                        ]
                    # k-outer: a slot's first KO matmuls only need the k=0
                    # activation tile (head pipelining with the z stream)
                    for k in range(KO):
                        for j2 in group:
                            emit_slot_matmuls(slot + j2, plan[j2], rs[j2],
                                              cur, k)
                    for j2 in group:
                        sl = slot + j2
                        sz = plan[j2]
                        last = nxt is None and j2 == len(plan) - 1
                        # stage-2 out tiles are per-slot, so eviction engines
                        # are unconstrained: for the FINAL slot let ACT own
                        # m1/m3 and issue their store itself (same-engine
                        # order, no sem round-trip) while SP stores m0/m2
                        emit_evictions(sl, s, j2, sz, rs[j2], nxt,
                                       act_parity=1 if last else 0)
                        if nxt is None:
                            dstv = outs[j2].rearrange("(ko p) c -> p ko c", p=P)
                            # final slot: ACT (which evicted m3 last) issues
                            # the store itself - no cross-engine sem wait
                            act_store = os.environ.get("MOE_ACT_STORE", "1") == "1"
                            eng = nc.scalar if (last and act_store) else nc.sync
                            eng.dma_start(dstv, out_tiles[j2][:, :, :sz])
                        del psums[sl]
                slot += len(plan)
    nc.compile()
    return nc


def _kernel_numpy_fallback(z, Ws, bs, ids_all):
    out = np.asarray(z, np.float32)
    for s in range(3):
        nxt = np.empty_like(out)
        ids = ids_all[s]
        for e in range(Ws[s].shape[0]):
            mask = ids == e
            if mask.any():
                nxt[mask] = np.maximum(out[mask] @ Ws[s][e] + bs[s][e], 0.0)
        out = nxt
    return out


def kernel(z, W_dataset, b_dataset, W_assay, b_assay, W_donor, b_donor,
           dataset_ids, assay_ids, donor_ids):
    global LAST_RESULTS
    import ml_dtypes

    mm_dtype_name = os.environ.get("MOE_MM_DTYPE", "bfloat16")
    np_mmdt = dict(bfloat16=ml_dtypes.bfloat16, float32r=np.float32,
                   float32=np.float32)[mm_dtype_name]

    ids_all = [
        np.asarray(dataset_ids, np.int32),
        np.asarray(assay_ids, np.int32),
        np.asarray(donor_ids, np.int32),
    ]
    Ws = [
        np.ascontiguousarray(np.asarray(W_dataset, np.float32)),
        np.ascontiguousarray(np.asarray(W_assay, np.float32)),
        np.ascontiguousarray(np.asarray(W_donor, np.float32)),
    ]
    bs = [
        np.asarray(b_dataset, np.float32),
        np.asarray(b_assay, np.float32),
        np.asarray(b_donor, np.float32),
    ]

    if any(np.any(np.diff(ids) < 0) for ids in ids_all):
        return _kernel_numpy_fallback(z, Ws, bs, ids_all)
    try:
        segs_all = [_segments(ids_all[s]) for s in range(3)]
        slot_plan, slots, share01 = make_plan(segs_all)
    except (AssertionError, IndexError):
        return _kernel_numpy_fallback(z, Ws, bs, ids_all)

    Ctot = sum(len(p) for p in slot_plan)

    # host-packed per-core weights/biases; lhsT layout per pack entry:
    # Wl[e][p, k*LD + m*P + j] = W[e][k*P + p, m*P + j]
    Wl = [
        Ws[s].reshape(STAGE_E[s], KO, P, KO, P)
        .transpose(0, 2, 1, 3, 4).reshape(STAGE_E[s], P, WSLOT).astype(np_mmdt)
        for s in range(3)
    ]
    Bl = [np.ascontiguousarray(bs[s].reshape(STAGE_E[s], KO, P).transpose(0, 2, 1))
          for s in range(3)]  # [E, P, KO]

    descs = np.zeros((NCORES, 1, Ctot), np.int32)
    Wpack = np.empty((NCORES, P, Ctot * WSLOT), np_mmdt)
    Bpack = np.empty((NCORES, P, Ctot * KO), np.float32)
    for c in range(NCORES):
        j = 0
        for s in range(3):
            for r, e, sz in slots[s][c]:
                descs[c, 0, j] = r - c * SH
                Wpack[c, :, j * WSLOT:(j + 1) * WSLOT] = Wl[s][e]
                Bpack[c, :, j * KO:(j + 1) * KO] = Bl[s][e]
                j += 1
        assert j == Ctot

    key = (slot_plan, share01, mm_dtype_name,
           os.environ.get("MOE_DESC_GPSIMD", "1"),
           os.environ.get("MOE_ACT_STORE", "1"))
    if key not in _program_cache:
        _program_cache[key] = _build_program(slot_plan, share01, mm_dtype_name)
    nc = _program_cache[key]

    zT = np.asarray(z, np.float32).T.astype(np_mmdt)  # [LD, N]
    in_maps = [
        {
            "zT_shard": np.ascontiguousarray(zT[:, c * SH:(c + 1) * SH]),
            "Wpack": Wpack[c],
            "Bpack": Bpack[c],
            "descs": descs[c],
        }
        for c in range(NCORES)
    ]

    res = bass_utils.run_bass_kernel_spmd(nc, in_maps, core_ids=list(range(NCORES)))
    LAST_RESULTS = res

    # assemble stage-2 windows in slot order, mirroring device write order
    out = np.empty((N, LD), np.float32)
    for c in range(NCORES):
        shard = out[c * SH:(c + 1) * SH]
        for j, (r, e, sz) in enumerate(slots[2][c]):
            shard[r - c * SH:r - c * SH + sz] = (
                res.results[c][f"outT_{j}"].astype(np.float32).T
            )
    return out


# revision 59
# speedup vs baseline: 1.6053x; 1.0051x over previous
"""Trainium2 Bass kernel for nn_DFVAE (3-stage MoE routing with sorted ids).

Strategy (hardcoded for N=16384, LD=512, experts (8, 6, 16), 8 cores):
  - Data-parallel: core c owns rows [2048c, 2048(c+1)).
  - bf16 activations + weights (f32 PSUM accumulate, f32 bias): ~4e-3 rel
    err end-to-end, well under the 2e-2 gate; halves all SBUF/HBM traffic
    and keeps full PE rate at any moving-dim size.
  - Sorted ids => contiguous expert pieces. Each (core, stage) is covered by
    a mixed grid of 256- and 512-row windows, each inside one piece:
    per piece floor(L/512) 512-windows + one overlap-shifted tail (256 if the
    remainder fits, else 512). Short shard-edge pieces get an edge-anchored
    spill window (min size covering the piece); every wrongly-spilled row is
    rewritten by the neighbor piece's first 512 head window because classes
    are emitted ascending (256 then 512) and shorts precede norms in-class.
    Cores pad to the per-stage class maxima with idempotent sub-window
    repeats, keeping the program uniform (SPMD).
  - Weights+biases are HOST-packed per core in slot order and DMA'd as plain
    static copies (no gpsimd gathers, no data-dependent descriptor
    generation); slot lhsT offsets are static. Padded slots duplicate.
  - Activations live as four per-k [P, SH] tiles per stage boundary; z
    streams as 4 k-chunk DMAs interleaved with W-slot-0's k-blocks. Every
    core's stage-0 early block holds >=2 windows of one expert, so slots 0,1
    share weight-pack entry 0 and run as a k-outer PAIR (8 PSUM banks =
    2 slots): ~1.7us of issuable PE work per z chunk covers the ~1.5us
    chunk arrival and the head runs stall-free from the first matmul.
  - Stage-2 outputs land in small per-slot tiles written at static offsets
    and stream to per-slot OUTPUT DRAM TENSORS via independent static DMAs
    (dynamic-dst stores to one tensor would serialize on write-write deps);
    the host assembles the final activation in slot order, mirroring device
    write order. The final 512 head window is split into two 256 halves and
    its store is issued by ACT itself, shrinking the tail chain.
  - Per-slot dynamic row offsets come from one batched TensorLoad per engine
    (PE/ACT/DVE); m-parity fixes the eviction engine so same-row rewrites
    stay ordered (same engine => program order).
  - A tiny warm-up matmul right after the descriptor DMA starts the PE
    p-state ramp clock early, so real matmuls run at the full 2.4 GHz.
"""
import os

import numpy as np

import concourse.bass as bass
import concourse.mybir as mybir
import concourse.tile as tile
from concourse import bacc, bass_utils
from concourse.bass import ds

N = 16384
LD = 512
NCORES = 8
SH = N // NCORES  # 2048 rows per core
P = 128
KO = LD // P  # 4 k/m subtiles
STAGE_E = (8, 6, 16)
SIZES = (256, 512)
WSLOT = KO * LD  # 2048 bf16 weight cols per pack entry per partition

LAST_RESULTS = None  # test harness reads exec_time_ns off this

_program_cache = {}


def _segments(ids):
    starts = np.flatnonzero(np.diff(ids, prepend=-1))
    ends = np.append(starts[1:], len(ids))
    return list(zip(starts.tolist(), ends.tolist(), ids[starts].tolist()))


def _core_windows(segs, lo, hi):
    """Per-class windows for one shard: {sz: (spills, norms)} lists of
    (start, expert). Raises on layouts this scheme can't cover."""
    big = max(SIZES)
    out = {sz: ([], []) for sz in SIZES}
    for a0, b0, e in segs:
        a, b = max(a0, lo), min(b0, hi)
        if a >= b:
            continue
        L = b - a
        edge = a == lo or b == hi
        if L < big:
            if not edge:
                raise AssertionError(f"interior short piece [{a},{b})")
            w = min(sz for sz in SIZES if sz >= L)
            out[w][0].append((lo if a == lo else hi - w, e))
            continue
        n, rem = L // big, L % big
        for i in range(n):
            out[big][1].append((a + i * big, e))
        if rem:
            t = min(sz for sz in SIZES if sz >= rem)  # t <= big <= L
            out[t][1].append((b - t, e))
    if not out[big][1]:
        raise AssertionError("no full-size in-piece window on shard")
    return out


def make_plan(segs_all):
    """Window layout for all (stage, core): returns (slot_plan, slots,
    share01) where slots[s][c] = [(start, expert, size), ...] in program
    order.

    Per stage the program slot sequence is
        [512]*(C512-H) ++ [small classes ascending] ++ [512]*H
    where H bounds the per-core count of spill-CONFLICTING 512 head windows
    (512 norms overlapping a spill window's range). Early 512 slots keep the
    PE fed at full rate while z streams in; spills sit in the small blocks
    and their conflicting heads come after, preserving overwrite correctness.
    All pads are idempotent repeats (non-conflicting writes of real windows)."""
    big = max(SIZES)
    allow_share = os.environ.get("MOE_SHARE01", "1") == "1"
    allow_split = os.environ.get("MOE_SPLIT_TAIL", "1") == "1"
    wins = [
        [_core_windows(segs_all[s], c * SH, (c + 1) * SH) for c in range(NCORES)]
        for s in range(3)
    ]
    slot_plan = []
    slots = []
    for s in range(3):
        C = {sz: max(len(wins[s][c][sz][0]) + len(wins[s][c][sz][1])
                     for c in range(NCORES)) for sz in SIZES}
        percore_cls = []
        for c in range(NCORES):
            w = wins[s][c]
            spills = [(r, e, sz) for sz in SIZES for r, e in w[sz][0]]
            big_norms = w[big][1]
            late = [
                (r, e) for r, e in big_norms
                if any(r < sr + ssz and sr < r + big for sr, _, ssz in spills)
            ]
            early = [(r, e) for r, e in big_norms if (r, e) not in late]
            small = {sz: list(w[sz][0]) + list(w[sz][1])
                     for sz in SIZES if sz != big}
            percore_cls.append((early, small, late, big_norms[-1]))
        H = max(len(late) for _, _, late, _ in percore_cls)
        smalls = tuple(sz for sz in sorted(SIZES) if sz != big)
        # stage 2 ends the kernel: split its final 512 window into two 256
        # halves so the tail eviction+store chain is half-sized (same rows)
        split_tail = s == 2 and H > 0 and allow_split
        plan = ((big,) * (C[big] - H)
                + tuple(sz for sz in smalls for _ in range(C[sz]))
                + (big,) * (H - 1 if split_tail else H)
                + ((256, 256) if split_tail else ()))
        slot_plan.append(plan)
        percore = []
        if s == 0:
            share01 = allow_share and C[big] - H >= 2
        for c in range(NCORES):
            early, small, late, pad = percore_cls[c]
            # move overflow norms into the late block (allowed: only heads
            # must come after spills; norms can go anywhere)
            while len(early) > C[big] - H:
                late.append(early.pop())
            while len(early) < C[big] - H:
                early.append(pad)
            while len(late) < H:
                late.append(pad)
            for sz in smalls:
                while len(small[sz]) < C[sz]:
                    small[sz].append((pad[0], pad[1]))  # sub-window: idem
                assert len(small[sz]) == C[sz]
            assert len(late) == H
            if s == 0 and share01:
                # put two same-expert windows first: slots 0 and 1 then share
                # weight-pack entry 0, so the opening slot-PAIR runs k-outer
                # without waiting for a second weight transfer
                pair = next(
                    ((i, jj) for i in range(len(early))
                     for jj in range(i + 1, len(early))
                     if early[i][1] == early[jj][1]),
                    None,
                )
                if pair is None:
                    share01 = False
                else:
                    i, jj = pair
                    early = ([early[i], early[jj]] +
                             [w for x, w in enumerate(early) if x not in (i, jj)])
            flat = ([(r, e, big) for r, e in early] +
                    [(r, e, sz) for sz in smalls for r, e in small[sz]] +
                    [(r, e, big) for r, e in late])
            if split_tail:
                r, e, _ = flat.pop()
                flat += [(r, e, 256), (r + 256, e, 256)]
            percore.append(flat)
        slots.append(percore)
    return tuple(slot_plan), slots, share01


def _build_program(slot_plan, share01, mm_dtype_name):
    nc = bacc.Bacc("TRN2", target_bir_lowering=False, debug=False,
                   enable_asserts=False, num_devices=NCORES)
    f32 = mybir.dt.float32
    i32 = mybir.dt.int32
    mmdt = getattr(mybir.dt, mm_dtype_name)
    ACT = mybir.EngineType.Activation
    DVE = mybir.EngineType.DVE
    PE = mybir.EngineType.PE
    Ctot = sum(len(p) for p in slot_plan)

    zT = nc.dram_tensor("zT_shard", [LD, SH], mmdt, kind="ExternalInput").ap()
    Wp = nc.dram_tensor("Wpack", [P, Ctot * WSLOT], mmdt, kind="ExternalInput").ap()
    Bp = nc.dram_tensor("Bpack", [P, Ctot * KO], f32, kind="ExternalInput").ap()
    desc_t = nc.dram_tensor("descs", [1, Ctot], i32, kind="ExternalInput").ap()
    outs = [
        nc.dram_tensor(f"outT_{j}", [LD, sz], mmdt, kind="ExternalOutput").ap()
        for j, sz in enumerate(slot_plan[2])
    ]

    with tile.TileContext(nc) as tc:
        with (
            tc.tile_pool(name="const", bufs=1) as cpool,
            tc.tile_pool(name="acts", bufs=1) as apool,
            tc.tile_pool(name="psum", bufs=8, space="PSUM") as ppool,
        ):
            desc_sb = cpool.tile([1, Ctot], i32)
            # gpsimd path: descgen on Pool, so the HWDGE descgen pipeline
            # starts on W0 immediately and the first transfers begin sooner
            if os.environ.get("MOE_DESC_GPSIMD", "1") == "1":
                nc.gpsimd.dma_start(desc_sb[:], desc_t)
            else:
                nc.sync.dma_start(desc_sb[:], desc_t)
            bias_sb = cpool.tile([P, Ctot * KO], f32)
            w_sb = cpool.tile([P, Ctot * WSLOT], mmdt)
            bufs = [
                [apool.tile([P, SH], mmdt, tag=f"act{s}_{k}", name=f"act{s}_{k}")
                 for k in range(KO)]
                for s in range(3)
            ]
            out_tiles = [
                apool.tile([P, KO, sz], mmdt, tag=f"out{j}", name=f"out{j}")
                for j, sz in enumerate(slot_plan[2])
            ]

            # DMA issue order = transfer order on the contended DMA engines:
            # the first slot-pair's weights, bias, the four z k-chunks
            # (the stage-0 pair consumes each chunk in ~1.7us >= its 1.5us
            # arrival), then the remaining slot packs stream under compute
            def _w_load(entry, n=1):
                nc.sync.dma_start(
                    w_sb[:, entry * WSLOT:(entry + n) * WSLOT],
                    Wp[:, entry * WSLOT:(entry + n) * WSLOT],
                )
            # interleave W0's k-blocks with the z k-chunks so the opening
            # pair's k-step matmuls unblock as each (weights, data) pair lands
            for k in range(KO):
                nc.sync.dma_start(w_sb[:, k * LD:(k + 1) * LD],
                                  Wp[:, k * LD:(k + 1) * LD])
                nc.sync.dma_start(bufs[0][k][:], zT[k * P:(k + 1) * P, :])
            nc.sync.dma_start(bias_sb[:], Bp)
            for j in range(1, Ctot):
                if j == 1 and share01:
                    continue  # slot 1 reads entry 0; entry 1 never read
                _w_load(j)

            # warm-up matmul: starts the PE p-state ramp clock early so real
            # matmuls hit full clock sooner (result never read)
            warm = ppool.tile([2, 16], f32, tag="ps", name="warm")
            nc.tensor.matmul(warm[:], lhsT=desc_sb.bitcast(mmdt)[0:1, 0:2],
                             rhs=desc_sb.bitcast(mmdt)[0:1, 0:16],
                             start=True, stop=True)

            rvals = nc.values_load_multi_w_load_instructions(
                desc_sb[0:1, 0:Ctot],
                engines=[PE, ACT, DVE],
                min_val=0,
                max_val=SH - min(SIZES),
                skip_runtime_bounds_check=True,
            )[1]

            def emit_slot_matmuls(slot, sz, r, cur, k):
                """One k-step (KO matmuls) of a slot into its 4 psum banks."""
                w_col = 0 if (slot == 1 and share01) else slot * WSLOT
                for m in range(KO):
                    nc.tensor.matmul(
                        psums[slot][m][:, :sz],
                        lhsT=w_sb[:, w_col + k * LD + m * P:
                                  w_col + k * LD + (m + 1) * P],
                        rhs=cur[k][:, ds(r, sz)],
                        start=(k == 0),
                        stop=(k == KO - 1),
                    )

            def emit_evictions(slot, s, j2, sz, r, nxt, act_parity=0):
                for m in range(KO):
                    psum = psums[slot][m]
                    bias_ap = bias_sb[:, slot * KO + m:slot * KO + m + 1]
                    dst = (nxt[m][:, ds(r, sz)] if nxt is not None
                           else out_tiles[j2][:, m, :sz])
                    if m % 2 != act_parity:
                        # relu(psum + b) on DVE: (psum + b) max 0; m-parity
                        # keeps each row's rewrites on one engine (ordered)
                        nc.vector.tensor_scalar(
                            dst, psum[:, :sz], bias_ap, 0.0,
                            mybir.AluOpType.add, mybir.AluOpType.max,
                        )
                    else:
                        nc.scalar.activation(
                            dst, psum[:, :sz],
                            mybir.ActivationFunctionType.Relu, bias=bias_ap,
                        )

            psums = {}
            slot = 0
            for s in range(3):
                cur = bufs[s]
                nxt = bufs[s + 1] if s < 2 else None
                plan = slot_plan[s]
                # when slots 0,1 share entry 0, run them as a k-outer PAIR
                # (8 psum banks): 1.7us of PE work per z k-chunk covers the
                # ~1.5us chunk arrival, so the z stream causes no stalls
                if s == 0 and share01:
                    groups = [[0, 1]] + [[j] for j in range(2, len(plan))]
                else:
                    groups = [[j] for j in range(len(plan))]
                for group in groups:
                    rs = {}
                    for j2 in group:
                        sl = slot + j2
                        sz = plan[j2]
                        rs[j2] = nc.s_assert_within(
                            rvals[sl], min_val=0, max_val=SH - sz,
                            skip_runtime_assert=True,
                        )
                        psums[sl] = [
                            ppool.tile([P, max(SIZES)], f32, tag="ps",
                                       name=f"ps{sl}_{m}")
                            for m in range(KO)
                        ]
                    # k-outer: a slot's first KO matmuls only need the k=0
                    # activation tile (head pipelining with the z stream)
                    for k in range(KO):
                        for j2 in group:
                            emit_slot_matmuls(slot + j2, plan[j2], rs[j2],
                                              cur, k)
                    for j2 in group:
                        sl = slot + j2
                        sz = plan[j2]
                        last = nxt is None and j2 == len(plan) - 1
                        # stage-2 out tiles are per-slot, so eviction engines
                        # are unconstrained: for the FINAL slot let ACT own
                        # m1/m3 and issue their store itself (same-engine
                        # order, no sem round-trip) while SP stores m0/m2
                        emit_evictions(sl, s, j2, sz, rs[j2], nxt,
                                       act_parity=1 if last else 0)
                        if nxt is None:
                            dstv = outs[j2].rearrange("(ko p) c -> p ko c", p=P)
                            # final slot: ACT (which evicted m3 last) issues
                            # the store itself - no cross-engine sem wait
                            act_store = os.environ.get("MOE_ACT_STORE", "1") == "1"
                            eng = nc.scalar if (last and act_store) else nc.sync
                            eng.dma_start(dstv, out_tiles[j2][:, :, :sz])
                        del psums[sl]
                slot += len(plan)
    nc.compile()
    return nc


def _kernel_numpy_fallback(z, Ws, bs, ids_all):
    out = np.asarray(z, np.float32)
    for s in range(3):
        nxt = np.empty_like(out)
        ids = ids_all[s]
        for e in range(Ws[s].shape[0]):
            mask = ids == e
            if mask.any():
                nxt[mask] = np.maximum(out[mask] @ Ws[s][e] + bs[s][e], 0.0)
        out = nxt
    return out


def kernel(z, W_dataset, b_dataset, W_assay, b_assay, W_donor, b_donor,
           dataset_ids, assay_ids, donor_ids):
    global LAST_RESULTS
    import ml_dtypes

    mm_dtype_name = os.environ.get("MOE_MM_DTYPE", "bfloat16")
    np_mmdt = dict(bfloat16=ml_dtypes.bfloat16, float32r=np.float32,
                   float32=np.float32)[mm_dtype_name]

    ids_all = [
        np.asarray(dataset_ids, np.int32),
        np.asarray(assay_ids, np.int32),
        np.asarray(donor_ids, np.int32),
    ]
    Ws = [
        np.ascontiguousarray(np.asarray(W_dataset, np.float32)),
        np.ascontiguousarray(np.asarray(W_assay, np.float32)),
        np.ascontiguousarray(np.asarray(W_donor, np.float32)),
    ]
    bs = [
        np.asarray(b_dataset, np.float32),
        np.asarray(b_assay, np.float32),
        np.asarray(b_donor, np.float32),
    ]

    if any(np.any(np.diff(ids) < 0) for ids in ids_all):
        return _kernel_numpy_fallback(z, Ws, bs, ids_all)
    try:
        segs_all = [_segments(ids_all[s]) for s in range(3)]
        slot_plan, slots, share01 = make_plan(segs_all)
    except (AssertionError, IndexError):
        return _kernel_numpy_fallback(z, Ws, bs, ids_all)

    Ctot = sum(len(p) for p in slot_plan)

    # host-packed per-core weights/biases; lhsT layout per pack entry:
    # Wl[e][p, k*LD + m*P + j] = W[e][k*P + p, m*P + j]
    Wl = [
        Ws[s].reshape(STAGE_E[s], KO, P, KO, P)
        .transpose(0, 2, 1, 3, 4).reshape(STAGE_E[s], P, WSLOT).astype(np_mmdt)
        for s in range(3)
    ]
    Bl = [np.ascontiguousarray(bs[s].reshape(STAGE_E[s], KO, P).transpose(0, 2, 1))
          for s in range(3)]  # [E, P, KO]

    descs = np.zeros((NCORES, 1, Ctot), np.int32)
    Wpack = np.empty((NCORES, P, Ctot * WSLOT), np_mmdt)
    Bpack = np.empty((NCORES, P, Ctot * KO), np.float32)
    for c in range(NCORES):
        j = 0
        for s in range(3):
            for r, e, sz in slots[s][c]:
                descs[c, 0, j] = r - c * SH
                Wpack[c, :, j * WSLOT:(j + 1) * WSLOT] = Wl[s][e]
                Bpack[c, :, j * KO:(j + 1) * KO] = Bl[s][e]
                j += 1
        assert j == Ctot

    key = (slot_plan, share01, mm_dtype_name,
           os.environ.get("MOE_DESC_GPSIMD", "1"),
           os.environ.get("MOE_ACT_STORE", "1"))
    if key not in _program_cache:
        _program_cache[key] = _build_program(slot_plan, share01, mm_dtype_name)
    nc = _program_cache[key]

    zT = np.asarray(z, np.float32).T.astype(np_mmdt)  # [LD, N]
    in_maps = [
        {
            "zT_shard": np.ascontiguousarray(zT[:, c * SH:(c + 1) * SH]),
            "Wpack": Wpack[c],
            "Bpack": Bpack[c],
            "descs": descs[c],
        }
        for c in range(NCORES)
    ]

    res = bass_utils.run_bass_kernel_spmd(nc, in_maps, core_ids=list(range(NCORES)))
    LAST_RESULTS = res

    # assemble stage-2 windows in slot order, mirroring device write order
    out = np.empty((N, LD), np.float32)
    for c in range(NCORES):
        shard = out[c * SH:(c + 1) * SH]
        for j, (r, e, sz) in enumerate(slots[2][c]):
            shard[r - c * SH:r - c * SH + sz] = (
                res.results[c][f"outT_{j}"].astype(np.float32).T
            )
    return out
